# revision 7
# baseline (speedup 1.0000x reference)
"""Trainium2 Bass kernel for per-sample AR(6) least-squares fit.

Problem (hardcoded): x [32768, 600] f32. For each sample independently:
  D = [1, x_lag1..x_lag6] over t=6..599 (T_out=594 rows, 7 cols)
  G = D^T D, b = D^T y (y = x[:, 6:]), w = solve(G, b)
  outputs: coeffs [N,600,6] = broadcast of w[1:7] (zero for t<6),
           p_logits [N,5] zeros, p_hard [N] int32 zeros,
           x_hat [N,600] = D @ w for t>=6, zero for t<6.

Sharding: pure data parallel, batch N split across 8 NeuronCores
(4096 samples per core). No collectives.

Device algorithm (per core, samples in SBUF partitions, 32 groups of 128):
  - G is a windowed autocorrelation matrix: only 7 lag-correlation base
    sums (fused tensor_tensor_reduce per group+lag) + cheap boundary-
    column chains are needed to fill all 7x8 augmented entries.
  - Batched (across groups) Gauss-Jordan elimination solves the 7x7
    systems with vector ops; G is SPD so no pivoting needed.
  - Predictions accumulate via fused scalar_tensor_tensor multiply-add.
  - The big coeffs output tile is materialized by the Scalar engine
    (broadcast copy) while the Vector engine computes, and everything
    overlaps with the output DMA stream.
"""
import numpy as np

import concourse.bass as bass
import concourse.tile as tile
from concourse import bacc, mybir
from concourse.bass_utils import run_bass_kernel_spmd

SEQ_LEN = 600
P = 6
T_OUT = SEQ_LEN - P  # 594
N_FULL = 32768
NCORES = 8
N_PER_CORE = N_FULL // NCORES  # 4096

F32 = mybir.dt.float32
BF16 = mybir.dt.bfloat16
I32 = mybir.dt.int32
Alu = mybir.AluOpType


def build_kernel(n_per_core=N_PER_CORE, groups_per_sweep=16, n_iters=1,
                 yhat_engine="pe"):
    """Build the per-core Bacc program. All cores run the same graph.

    n_iters > 1 wraps the body in a dynamic loop (benchmark builds only).
    """
    assert n_per_core % 128 == 0
    n_groups = n_per_core // 128
    gs = min(groups_per_sweep, n_groups)
    assert n_groups % gs == 0
    n_sweeps = n_groups // gs

    nc = bacc.Bacc("TRN2", target_bir_lowering=False, debug=False,
                   num_devices=NCORES)
    x_in = nc.dram_tensor("x", [n_per_core, SEQ_LEN], F32, kind="ExternalInput")
    coeffs_out = nc.dram_tensor("coeffs", [n_per_core, SEQ_LEN, P], F32,
                                kind="ExternalOutput")
    p_logits_out = nc.dram_tensor("p_logits", [n_per_core, 5], F32,
                                  kind="ExternalOutput")
    p_hard_out = nc.dram_tensor("p_hard", [n_per_core], I32,
                                kind="ExternalOutput")
    x_hat_out = nc.dram_tensor("x_hat", [n_per_core, SEQ_LEN], F32,
                               kind="ExternalOutput")

    with tile.TileContext(nc) as tc:
        with (
            tc.tile_pool(name="xpool", bufs=2) as xpool,
            tc.tile_pool(name="small", bufs=2) as small,
            tc.tile_pool(name="scratch", bufs=2) as scratch,
            tc.tile_pool(name="outp", bufs=3) as outp,
            tc.tile_pool(name="zpool", bufs=1) as zpool,
            tc.tile_pool(name="psum", bufs=2, space="PSUM") as psum,
        ):
            def body():
                # constant zero tiles for p_logits / p_hard
                zl = zpool.tile([128, n_groups, 5], F32, name="zl")
                nc.vector.memset(zl[:], 0.0)
                nc.sync.dma_start(
                    out=p_logits_out.rearrange("(g p) c -> p g c", p=128),
                    in_=zl[:])
                zi = zpool.tile([128, n_groups], I32, name="zi")
                nc.vector.memset(zi[:], 0)
                nc.sync.dma_start(
                    out=p_hard_out.rearrange("(g p) -> p g", p=128),
                    in_=zi[:])
                if yhat_engine == "pe":
                    ones_bf = zpool.tile([128, T_OUT], BF16, name="ones_bf")
                    nc.vector.memset(ones_bf[:], 1.0)
                else:
                    ones_bf = None

                for sw in range(n_sweeps):
                    sweep(sw, ones_bf)

            def sweep(sw, ones_bf):
                base_g = sw * gs
                # ---- load x for this sweep, one DMA per group ----
                x_sb = xpool.tile([128, gs, SEQ_LEN], F32, name="x_sb", tag="x_sb")
                for g in range(gs):
                    r0 = (base_g + g) * 128
                    nc.scalar.dma_start(out=x_sb[:, g, :],
                                        in_=x_in[r0:r0 + 128, :])
                if yhat_engine == "pe":
                    # bf16 copy of x for the TensorEngine prediction pass
                    x_bf = xpool.tile([128, gs, SEQ_LEN], BF16, name="x_bf",
                                      tag="x_bf")
                    for g in range(gs):
                        nc.scalar.copy(x_bf[:, g, :], x_sb[:, g, :])

                def col(s):
                    return x_sb[:, :, s]  # [128, gs]

                # ---- lag-correlation base sums via fused TT+reduce ----
                # corr[d, g] = sum over base window of x[s]*x[s+d]
                corr = small.tile([128, 7, gs], F32, name="corr", tag="corr")
                for d in range(7):
                    if d < 6:
                        lo = 5 - d  # window [5-d, 598-d]
                    else:
                        lo = 0      # d=6: window [0, 593] (the b[6] window)
                    for g in range(gs):
                        tout = scratch.tile([128, T_OUT], F32, name="tout",
                                            tag="tout")
                        # fused product+reduce: out = (in0*1)*in1,
                        # accum_out = sum(out)
                        nc.vector.scalar_tensor_tensor(
                            out=tout[:],
                            in0=x_sb[:, g, lo:lo + T_OUT],
                            scalar=1.0,
                            in1=x_sb[:, g, lo + d:lo + d + T_OUT],
                            op0=Alu.mult,
                            op1=Alu.mult,
                            accum_out=corr[:, d, g:g + 1],
                        )

                # ---- plain window sums S_0 = sum x[5..598] per group ----
                ssum = small.tile([128, gs], F32, name="ssum", tag="ssum")
                for g in range(gs):
                    tout2 = scratch.tile([128, T_OUT], F32, name="tout2",
                                         tag="tout")
                    nc.vector.tensor_scalar(
                        out=tout2[:],
                        in0=x_sb[:, g, 5:5 + T_OUT],
                        scalar1=1.0,
                        scalar2=None,
                        op0=Alu.mult,
                        op1=Alu.add,
                        accum_out=ssum[:, g:g + 1],
                    )

                # ---- assemble augmented systems AUG [128, gs, 7, 8] ----
                # rows i=0..6 of [G | b]; col 7 is b.
                aug = small.tile([128, gs, 7, 8], F32, name="aug", tag="aug")
                sc1 = small.tile([128, gs], F32, name="sc1", tag="sc1")
                sc2 = small.tile([128, gs], F32, name="sc2", tag="sc2")

                nc.vector.memset(aug[:, :, 0, 0], float(T_OUT))
                # S chain: row 0 cols 1..6;  S_b = sum x[5-b .. 598-b]
                nc.vector.tensor_copy(aug[:, :, 0, 1], ssum[:])
                for b in range(1, 6):
                    nc.vector.tensor_add(aug[:, :, 0, 1 + b],
                                         aug[:, :, 0, b], col(5 - b))
                    nc.vector.tensor_sub(aug[:, :, 0, 1 + b],
                                         aug[:, :, 0, 1 + b], col(599 - b))
                # b[0] = sum x[6..599] = S_0 - x[5] + x[599]
                nc.vector.tensor_sub(aug[:, :, 0, 7], ssum[:], col(5))
                nc.vector.tensor_add(aug[:, :, 0, 7], aug[:, :, 0, 7], col(599))

                # C diagonals: base C_{0,d} = corr[d] at G[1][1+d]
                for d in range(6):
                    nc.vector.tensor_copy(aug[:, :, 1, 1 + d], corr[:, d, :])
                    for j in range(1, 6 - d):
                        # C_{j,j+d} = C_{j-1,j-1+d} + p_d[5-j-d] - p_d[599-j-d]
                        nc.vector.tensor_mul(sc1[:], col(5 - j - d), col(5 - j))
                        nc.vector.tensor_add(aug[:, :, 1 + j, 1 + j + d],
                                             aug[:, :, j, j + d], sc1[:])
                        nc.vector.tensor_mul(sc2[:], col(599 - j - d),
                                             col(599 - j))
                        nc.vector.tensor_sub(aug[:, :, 1 + j, 1 + j + d],
                                             aug[:, :, 1 + j, 1 + j + d],
                                             sc2[:])

                # b entries: row 1+a col 7, lag d=a+1
                for d in range(1, 6):
                    # b[d] = corr[d] - p_d[5-d] + p_d[599-d]
                    nc.vector.tensor_mul(sc1[:], col(5 - d), col(5))
                    nc.vector.tensor_sub(aug[:, :, d, 7], corr[:, d, :], sc1[:])
                    nc.vector.tensor_mul(sc2[:], col(599 - d), col(599))
                    nc.vector.tensor_add(aug[:, :, d, 7], aug[:, :, d, 7],
                                         sc2[:])
                nc.vector.tensor_copy(aug[:, :, 6, 7], corr[:, 6, :])

                # mirrors: G[1+a][0] = S_a  (strided copy of row 0 cols 1..6)
                nc.any.tensor_copy(aug[:, :, 1:7, 0], aug[:, :, 0, 1:7])
                # C mirrors below diagonal
                for d in range(1, 6):
                    for j in range(6 - d):
                        nc.any.tensor_copy(aug[:, :, 1 + j + d, 1 + j],
                                           aug[:, :, 1 + j, 1 + j + d])

                # ---- batched Gauss-Jordan (no pivoting; G is SPD) ----
                rcp = small.tile([128, gs], F32, name="rcp", tag="rcp")
                etmp = small.tile([128, gs, 6, 8], F32, name="etmp", tag="etmp")
                for k in range(7):
                    nc.vector.reciprocal(rcp[:], aug[:, :, k, k])
                    nc.vector.tensor_mul(
                        aug[:, :, k, k:8], aug[:, :, k, k:8],
                        rcp.unsqueeze(2).broadcast_to([128, gs, 8 - k]))
                    if k < 6:
                        nrows = 6 - k
                        mult_b = (aug[:, :, k + 1:7, k].unsqueeze(3)
                                  .broadcast_to([128, gs, nrows, 8 - k]))
                        prow_b = (aug[:, :, k, k:8].unsqueeze(2)
                                  .broadcast_to([128, gs, nrows, 8 - k]))
                        nc.vector.tensor_mul(etmp[:, :, :nrows, :8 - k],
                                             mult_b, prow_b)
                        nc.vector.tensor_sub(aug[:, :, k + 1:7, k:8],
                                             aug[:, :, k + 1:7, k:8],
                                             etmp[:, :, :nrows, :8 - k])
                # back substitution on the augmented column
                for k in range(6, 0, -1):
                    wk = (aug[:, :, k, 7].unsqueeze(2)
                          .broadcast_to([128, gs, k]))
                    nc.vector.tensor_mul(etmp[:, :, 0, :k],
                                         aug[:, :, 0:k, k], wk)
                    nc.vector.tensor_sub(aug[:, :, 0:k, 7],
                                         aug[:, :, 0:k, 7],
                                         etmp[:, :, 0, :k])
                # solution: w_i = aug[:, :, i, 7]

                if yhat_engine == "pe":
                    # bf16 copy of w for diag matrices / PE
                    w_bf = small.tile([128, gs, 7], BF16, name="w_bf",
                                      tag="w_bf")
                    nc.vector.tensor_copy(w_bf[:], aug[:, :, :, 7])
                    # diag matrices dg[p, g, i, j] = w_i[g](p) if j==p else 0
                    dg = small.tile([128, gs, 7, 128], BF16, name="dg",
                                    tag="dg")
                    nc.gpsimd.affine_select(
                        out=dg[:],
                        in_=w_bf.unsqueeze(3).broadcast_to([128, gs, 7, 128]),
                        pattern=[[0, gs], [0, 7], [1, 128]],
                        compare_op=Alu.is_equal,
                        fill=0.0,
                        base=0,
                        channel_multiplier=-1,
                    )

                # ---- per-group outputs ----
                NA = 512  # PSUM bank column split
                for g in range(gs):
                    r0 = (base_g + g) * 128
                    # x_hat: y = w0 + sum_a w_{1+a} * x[5-a .. 598-a]
                    y = outp.tile([128, SEQ_LEN], F32, name="y", tag="y")
                    nc.vector.memset(y[:, 0:P], 0.0)
                    if yhat_engine == "pe":
                        # y[t] = sum_i diag(w_i) @ x_window_i  in PSUM
                        ypa = psum.tile([128, NA], F32, name="ypa", tag="ypa")
                        ypb = psum.tile([128, T_OUT - NA], F32, name="ypb",
                                        tag="ypb")
                        for term in range(7):
                            if term == 0:
                                lhsT = dg[:, g, 0, :]
                                rA = ones_bf[:, 0:NA]
                                rB = ones_bf[:, NA:T_OUT]
                            else:
                                a = term - 1
                                lhsT = dg[:, g, 1 + a, :]
                                lo = 5 - a
                                rA = x_bf[:, g, lo:lo + NA]
                                rB = x_bf[:, g, lo + NA:lo + T_OUT]
                            st = term == 0
                            sp = term == 6
                            nc.tensor.matmul(ypa[:], lhsT, rA, start=st,
                                             stop=sp)
                            nc.tensor.matmul(ypb[:], lhsT, rB, start=st,
                                             stop=sp)
                        nc.scalar.copy(y[:, P:P + NA], ypa[:])
                        nc.scalar.copy(y[:, P + NA:SEQ_LEN], ypb[:])
                    else:
                        nc.vector.tensor_scalar(
                            out=y[:, P:SEQ_LEN],
                            in0=x_sb[:, g, P:SEQ_LEN],
                            scalar1=0.0,
                            scalar2=aug[:, g, 0, 7:8],
                            op0=Alu.mult,
                            op1=Alu.add,
                        )
                        for a in range(6):
                            nc.vector.scalar_tensor_tensor(
                                out=y[:, P:SEQ_LEN],
                                in0=x_sb[:, g, 5 - a:599 - a],
                                scalar=aug[:, g, 1 + a, 7:8],
                                in1=y[:, P:SEQ_LEN],
                                op0=Alu.mult,
                                op1=Alu.add,
                            )
                    nc.sync.dma_start(out=x_hat_out[r0:r0 + 128, :], in_=y[:])

                    # coeffs: broadcast w[1:7] across t (zero t<6)
                    cb = outp.tile([128, SEQ_LEN, P], F32, name="cb", tag="cb")
                    nc.vector.memset(cb[:, 0:P, :], 0.0)
                    war = aug[:, g, 1:7, 7]  # [128, 6]
                    nc.scalar.copy(
                        cb[:, P:SEQ_LEN, :],
                        war.unsqueeze(1).broadcast_to([128, T_OUT, 6]))
                    nc.sync.dma_start(out=coeffs_out[r0:r0 + 128], in_=cb[:])

            if n_iters == 1:
                body()
            else:
                with tc.For_i(0, n_iters, 1):
                    body()

    nc.compile()
    return nc


def kernel(x: np.ndarray):
    """Full-input entry point. Shards across 8 cores, runs, gathers."""
    assert x.shape == (N_FULL, SEQ_LEN) and x.dtype == np.float32
    nc = build_kernel()
    in_maps = [
        {"x": np.ascontiguousarray(x[c * N_PER_CORE:(c + 1) * N_PER_CORE])}
        for c in range(NCORES)
    ]
    res = run_bass_kernel_spmd(nc, in_maps, core_ids=list(range(NCORES)))
    coeffs = np.concatenate([res.results[c]["coeffs"] for c in range(NCORES)])
    p_logits = np.concatenate([res.results[c]["p_logits"] for c in range(NCORES)])
    p_hard = np.concatenate([res.results[c]["p_hard"] for c in range(NCORES)])
    x_hat = np.concatenate([res.results[c]["x_hat"] for c in range(NCORES)])
    return coeffs, p_logits, p_hard, x_hat


# revision 14
# speedup vs baseline: 223.4634x; 223.4634x over previous
"""Trainium2 Bass kernel for per-sample AR(6) least-squares fit.

Problem (hardcoded): x [32768, 600] f32. For each sample independently:
  D = [1, x_lag1..x_lag6] over t=6..599 (T_out=594 rows, 7 cols)
  G = D^T D, b = D^T y (y = x[:, 6:]), w = solve(G, b)
  outputs: coeffs [N,600,6] = broadcast of w[1:7] (zero for t<6),
           p_logits [N,5] zeros, p_hard [N] int32 zeros,
           x_hat [N,600] = D @ w for t>=6, zero for t<6.

Sharding: pure data parallel, batch N split across 8 NeuronCores
(4096 samples per core). No collectives.

Device algorithm (per core, samples in SBUF partitions, 32 groups of 128):
  - G is a windowed autocorrelation matrix: only 7 lag-correlation base
    sums (fused tensor_tensor_reduce per group+lag) + cheap boundary-
    column chains are needed to fill all 7x8 augmented entries.
  - Batched (across groups) Gauss-Jordan elimination solves the 7x7
    systems with vector ops; G is SPD so no pivoting needed.
  - Predictions accumulate via fused scalar_tensor_tensor multiply-add.
  - The big coeffs output tile is materialized by the Scalar engine
    (broadcast copy) while the Vector engine computes, and everything
    overlaps with the output DMA stream.
"""
import numpy as np

import concourse.bass as bass
import concourse.tile as tile
from concourse import bacc, mybir
from concourse.bass_utils import run_bass_kernel_spmd

SEQ_LEN = 600
P = 6
T_OUT = SEQ_LEN - P  # 594
N_FULL = 32768
NCORES = 8
N_PER_CORE = N_FULL // NCORES  # 4096

F32 = mybir.dt.float32
BF16 = mybir.dt.bfloat16
I32 = mybir.dt.int32
Alu = mybir.AluOpType


def build_kernel(n_per_core=N_PER_CORE, groups_per_sweep=8, n_iters=1,
                 yhat_engine="pe", bench=False):
    """Build the per-core Bacc program. All cores run the same graph.

    n_iters > 1 wraps the body in a dynamic loop (benchmark builds only).
    bench=True makes all big I/O Internal DRAM (no host transfers) and
    exposes only a tiny probe output — used for wall-clock slope timing.
    """
    assert n_per_core % 128 == 0
    n_groups = n_per_core // 128
    gs = min(groups_per_sweep, n_groups)
    assert n_groups % gs == 0
    n_sweeps = n_groups // gs

    nc = bacc.Bacc("TRN2", target_bir_lowering=False, debug=False,
                   num_devices=NCORES)
    io_kind = "Internal" if bench else "ExternalInput"
    out_kind = "Internal" if bench else "ExternalOutput"
    x_in = nc.dram_tensor("x", [n_per_core, SEQ_LEN], F32, kind=io_kind)
    coeffs_out = nc.dram_tensor("coeffs", [n_per_core, SEQ_LEN, P], F32,
                                kind=out_kind)
    p_logits_out = nc.dram_tensor("p_logits", [n_per_core, 5], F32,
                                  kind=out_kind)
    p_hard_out = nc.dram_tensor("p_hard", [n_per_core], I32,
                                kind=out_kind)
    x_hat_out = nc.dram_tensor("x_hat", [n_per_core, SEQ_LEN], F32,
                               kind=out_kind)
    probe_out = None
    if bench:
        probe_out = nc.dram_tensor("probe", [128, 8], F32,
                                   kind="ExternalOutput")

    with tile.TileContext(nc) as tc:
        with (
            tc.tile_pool(name="xpool", bufs=2) as xpool,
            tc.tile_pool(name="small", bufs=2) as small,
            tc.tile_pool(name="scratch", bufs=2) as scratch,
            tc.tile_pool(name="outp", bufs=3) as outp,
            tc.tile_pool(name="zpool", bufs=1) as zpool,
            tc.tile_pool(name="psum", bufs=2, space="PSUM") as psum,
        ):
            def body():
                # constant zero tiles for p_logits / p_hard
                zl = zpool.tile([128, n_groups, 5], F32, name="zl")
                nc.vector.memset(zl[:], 0.0)
                nc.sync.dma_start(
                    out=p_logits_out.rearrange("(g p) c -> p g c", p=128),
                    in_=zl[:])
                zi = zpool.tile([128, n_groups], I32, name="zi")
                nc.vector.memset(zi[:], 0)
                nc.sync.dma_start(
                    out=p_hard_out.rearrange("(g p) -> p g", p=128),
                    in_=zi[:])
                if yhat_engine == "pe":
                    ones_bf = zpool.tile([128, T_OUT], BF16, name="ones_bf")
                    nc.vector.memset(ones_bf[:], 1.0)
                else:
                    ones_bf = None

                for sw in range(n_sweeps):
                    sweep(sw, ones_bf)

            def sweep(sw, ones_bf):
                base_g = sw * gs
                # ---- load x for this sweep, 4 groups per DMA (~1.2MB) ----
                x_sb = xpool.tile([128, gs, SEQ_LEN], F32, name="x_sb", tag="x_sb")
                lb = min(4, gs)
                for g in range(0, gs, lb):
                    r0 = (base_g + g) * 128
                    nc.scalar.dma_start(
                        out=x_sb[:, g:g + lb, :],
                        in_=x_in[r0:r0 + lb * 128, :].rearrange(
                            "(g p) t -> p g t", p=128))
                if yhat_engine == "pe":
                    # bf16 copy of x for the TensorEngine prediction pass
                    x_bf = xpool.tile([128, gs, SEQ_LEN], BF16, name="x_bf",
                                      tag="x_bf")
                    for g in range(gs):
                        nc.scalar.copy(x_bf[:, g, :], x_sb[:, g, :])

                def col(s):
                    return x_sb[:, :, s]  # [128, gs]

                # ---- lag-correlation base sums via fused TT+reduce ----
                # corr[g, d] = sum over base window of x[s]*x[s+d]
                corr = small.tile([128, gs, 7], F32, name="corr", tag="corr")
                for d in range(7):
                    if d < 6:
                        lo = 5 - d  # window [5-d, 598-d]
                    else:
                        lo = 0      # d=6: window [0, 593] (the b[6] window)
                    for g in range(gs):
                        tout = scratch.tile([128, T_OUT], F32, name="tout",
                                            tag="tout")
                        # fused product+reduce: out = (in0*1)*in1,
                        # accum_out = sum(out)
                        nc.vector.scalar_tensor_tensor(
                            out=tout[:],
                            in0=x_sb[:, g, lo:lo + T_OUT],
                            scalar=1.0,
                            in1=x_sb[:, g, lo + d:lo + d + T_OUT],
                            op0=Alu.mult,
                            op1=Alu.mult,
                            accum_out=corr[:, g, d:d + 1],
                        )

                # ---- plain window sums S_0 = sum x[5..598] per group ----
                ssum = small.tile([128, gs], F32, name="ssum", tag="ssum")
                for g in range(gs):
                    tout2 = scratch.tile([128, T_OUT], F32, name="tout2",
                                         tag="tout")
                    nc.vector.tensor_scalar(
                        out=tout2[:],
                        in0=x_sb[:, g, 5:5 + T_OUT],
                        scalar1=1.0,
                        scalar2=None,
                        op0=Alu.mult,
                        op1=Alu.add,
                        accum_out=ssum[:, g:g + 1],
                    )

                # ---- assemble augmented systems AUG [128, gs, 7, 8] ----
                # rows i=0..6 of [G | b]; col 7 is b. All chain/boundary ops
                # are batched across lags via strided/reversed APs.
                aug = small.tile([128, gs, 7, 8], F32, name="aug", tag="aug")
                sc1 = small.tile([128, gs, 5], F32, name="sc1", tag="sc1")
                sc2 = small.tile([128, gs, 5], F32, name="sc2", tag="sc2")

                def bcol(s, m):
                    # x[:, :, s] broadcast along a new trailing dim of size m
                    return (x_sb[:, :, s].unsqueeze(2)
                            .broadcast_to([128, gs, m]))

                nc.vector.memset(aug[:, :, 0, 0], float(T_OUT))
                # S chain: row 0 cols 1..6;  S_b = sum x[5-b .. 598-b]
                # deltas_b = x[5-b] - x[599-b]  (b=1..5) in one op
                nc.vector.tensor_sub(sc1[:, :, 0:5],
                                     x_sb[:, :, 4::-1][:, :, 0:5],
                                     x_sb[:, :, 598::-1][:, :, 0:5])
                nc.vector.tensor_copy(aug[:, :, 0, 1], ssum[:])
                for b in range(1, 6):
                    nc.vector.tensor_add(aug[:, :, 0, 1 + b],
                                         aug[:, :, 0, b], sc1[:, :, b - 1])
                # b[0] = sum x[6..599] = S_0 - x[5] + x[599]
                nc.vector.tensor_sub(aug[:, :, 0, 7], ssum[:], col(5))
                nc.vector.tensor_add(aug[:, :, 0, 7], aug[:, :, 0, 7], col(599))

                # C diagonals: bases C_{0,d} = corr[d] land at G[1][1+d]
                nc.vector.tensor_copy(aug[:, :, 1, 1:7], corr[:, :, 0:6])
                # chain step j (all alive lags at once):
                # C_{j,j+d} = C_{j-1,j-1+d} + p_d[5-j-d] - p_d[599-j-d]
                for j in range(1, 6):
                    m = 6 - j  # alive lags d = 0..m-1
                    nc.vector.tensor_mul(sc1[:, :, 0:m],
                                         x_sb[:, :, 5 - j::-1][:, :, 0:m],
                                         bcol(5 - j, m))
                    nc.vector.tensor_mul(sc2[:, :, 0:m],
                                         x_sb[:, :, 599 - j::-1][:, :, 0:m],
                                         bcol(599 - j, m))
                    nc.vector.tensor_add(aug[:, :, 1 + j, 1 + j:7],
                                         aug[:, :, j, j:6], sc1[:, :, 0:m])
                    nc.vector.tensor_sub(aug[:, :, 1 + j, 1 + j:7],
                                         aug[:, :, 1 + j, 1 + j:7],
                                         sc2[:, :, 0:m])

                # b entries (rows 1..5, col 7), lag d = row:
                # b[d] = corr[d] - x[5-d]*x[5] + x[599-d]*x[599]
                nc.vector.tensor_mul(sc1[:, :, 0:5],
                                     x_sb[:, :, 4::-1][:, :, 0:5], bcol(5, 5))
                nc.vector.tensor_mul(sc2[:, :, 0:5],
                                     x_sb[:, :, 598::-1][:, :, 0:5],
                                     bcol(599, 5))
                nc.vector.tensor_sub(aug[:, :, 1:6, 7], corr[:, :, 1:6],
                                     sc1[:, :, 0:5])
                nc.vector.tensor_add(aug[:, :, 1:6, 7], aug[:, :, 1:6, 7],
                                     sc2[:, :, 0:5])
                nc.vector.tensor_copy(aug[:, :, 6, 7], corr[:, :, 6])

                # mirrors: G[1+a][0] = S_a  (strided copy of row 0 cols 1..6)
                av = aug.rearrange("p g i j -> p g (i j)")
                nc.any.tensor_copy(av[:, :, 8:56:8], av[:, :, 1:7])
                # C mirrors below diagonal: one stride-9 diagonal copy per d
                for d in range(1, 6):
                    m = 6 - d
                    nc.any.tensor_copy(
                        av[:, :, 9 + 8 * d::9][:, :, 0:m],
                        av[:, :, 9 + d::9][:, :, 0:m])

                # ---- batched Gauss-Jordan (no pivoting; G is SPD) ----
                rcp = small.tile([128, gs], F32, name="rcp", tag="rcp")
                etmp = small.tile([128, gs, 6, 8], F32, name="etmp", tag="etmp")
                for k in range(7):
                    nc.vector.reciprocal(rcp[:], aug[:, :, k, k])
                    nc.vector.tensor_mul(
                        aug[:, :, k, k:8], aug[:, :, k, k:8],
                        rcp.unsqueeze(2).broadcast_to([128, gs, 8 - k]))
                    if k < 6:
                        nrows = 6 - k
                        mult_b = (aug[:, :, k + 1:7, k].unsqueeze(3)
                                  .broadcast_to([128, gs, nrows, 8 - k]))
                        prow_b = (aug[:, :, k, k:8].unsqueeze(2)
                                  .broadcast_to([128, gs, nrows, 8 - k]))
                        nc.vector.tensor_mul(etmp[:, :, :nrows, :8 - k],
                                             mult_b, prow_b)
                        nc.vector.tensor_sub(aug[:, :, k + 1:7, k:8],
                                             aug[:, :, k + 1:7, k:8],
                                             etmp[:, :, :nrows, :8 - k])
                # back substitution on the augmented column
                for k in range(6, 0, -1):
                    wk = (aug[:, :, k, 7].unsqueeze(2)
                          .broadcast_to([128, gs, k]))
                    nc.vector.tensor_mul(etmp[:, :, 0, :k],
                                         aug[:, :, 0:k, k], wk)
                    nc.vector.tensor_sub(aug[:, :, 0:k, 7],
                                         aug[:, :, 0:k, 7],
                                         etmp[:, :, 0, :k])
                # solution: w_i = aug[:, :, i, 7]

                if yhat_engine == "pe":
                    # bf16 copy of w for diag matrices / PE
                    w_bf = small.tile([128, gs, 7], BF16, name="w_bf",
                                      tag="w_bf")
                    nc.vector.tensor_copy(w_bf[:], aug[:, :, :, 7])
                    # diag matrices dg[p, g, i, j] = w_i[g](p) if j==p else 0
                    dg = small.tile([128, gs, 7, 128], BF16, name="dg",
                                    tag="dg")
                    nc.gpsimd.affine_select(
                        out=dg[:],
                        in_=w_bf.unsqueeze(3).broadcast_to([128, gs, 7, 128]),
                        pattern=[[0, gs], [0, 7], [1, 128]],
                        compare_op=Alu.is_equal,
                        fill=0.0,
                        base=0,
                        channel_multiplier=-1,
                    )

                # ---- per-group outputs ----
                NA = 512  # PSUM bank column split
                yb = min(4, gs)  # x_hat groups per DMA
                y4 = None
                for g in range(gs):
                    r0 = (base_g + g) * 128
                    gi = g % yb
                    if gi == 0:
                        y4 = outp.tile([128, yb, SEQ_LEN], F32, name="y4",
                                       tag="y4")
                        nc.any.memset(y4[:, :, 0:P], 0.0)
                    y = y4[:, gi, :]
                    if yhat_engine == "pe":
                        # y[t] = sum_i diag(w_i) @ x_window_i  in PSUM
                        ypa = psum.tile([128, NA], F32, name="ypa", tag="ypa")
                        ypb = psum.tile([128, T_OUT - NA], F32, name="ypb",
                                        tag="ypb")
                        for term in range(7):
                            if term == 0:
                                lhsT = dg[:, g, 0, :]
                                rA = ones_bf[:, 0:NA]
                                rB = ones_bf[:, NA:T_OUT]
                            else:
                                a = term - 1
                                lhsT = dg[:, g, 1 + a, :]
                                lo = 5 - a
                                rA = x_bf[:, g, lo:lo + NA]
                                rB = x_bf[:, g, lo + NA:lo + T_OUT]
                            st = term == 0
                            sp = term == 6
                            nc.tensor.matmul(ypa[:], lhsT, rA, start=st,
                                             stop=sp)
                            nc.tensor.matmul(ypb[:], lhsT, rB, start=st,
                                             stop=sp)
                        nc.scalar.copy(y[:, P:P + NA], ypa[:])
                        nc.scalar.copy(y[:, P + NA:SEQ_LEN], ypb[:])
                    else:
                        nc.vector.tensor_scalar(
                            out=y[:, P:SEQ_LEN],
                            in0=x_sb[:, g, P:SEQ_LEN],
                            scalar1=0.0,
                            scalar2=aug[:, g, 0, 7:8],
                            op0=Alu.mult,
                            op1=Alu.add,
                        )
                        for a in range(6):
                            nc.vector.scalar_tensor_tensor(
                                out=y[:, P:SEQ_LEN],
                                in0=x_sb[:, g, 5 - a:599 - a],
                                scalar=aug[:, g, 1 + a, 7:8],
                                in1=y[:, P:SEQ_LEN],
                                op0=Alu.mult,
                                op1=Alu.add,
                            )
                    if gi == yb - 1:
                        rb = (base_g + g - gi) * 128
                        nc.sync.dma_start(
                            out=x_hat_out[rb:rb + yb * 128, :].rearrange(
                                "(g p) t -> p g t", p=128),
                            in_=y4[:])

                    # coeffs: broadcast w[1:7] across t (zero t<6)
                    cb = outp.tile([128, SEQ_LEN, P], F32, name="cb", tag="cb")
                    nc.any.memset(cb[:, 0:P, :], 0.0)
                    war = aug[:, g, 1:7, 7]  # [128, 6]
                    nc.scalar.copy(
                        cb[:, P:SEQ_LEN, :],
                        war.unsqueeze(1).broadcast_to([128, T_OUT, 6]))
                    nc.sync.dma_start(out=coeffs_out[r0:r0 + 128], in_=cb[:])

            if n_iters == 1:
                body()
            else:
                with tc.For_i(0, n_iters, 1):
                    body()
            if bench:
                pr = zpool.tile([128, 8], F32, name="pr")
                nc.vector.memset(pr[:], 1.0)
                nc.sync.dma_start(out=probe_out[:, :], in_=pr[:])

    nc.compile()
    return nc


def kernel(x: np.ndarray):
    """Full-input entry point. Shards across 8 cores, runs, gathers."""
    assert x.shape == (N_FULL, SEQ_LEN) and x.dtype == np.float32
    nc = build_kernel()
    in_maps = [
        {"x": np.ascontiguousarray(x[c * N_PER_CORE:(c + 1) * N_PER_CORE])}
        for c in range(NCORES)
    ]
    res = run_bass_kernel_spmd(nc, in_maps, core_ids=list(range(NCORES)))
    coeffs = np.concatenate([res.results[c]["coeffs"] for c in range(NCORES)])
    p_logits = np.concatenate([res.results[c]["p_logits"] for c in range(NCORES)])
    p_hard = np.concatenate([res.results[c]["p_hard"] for c in range(NCORES)])
    x_hat = np.concatenate([res.results[c]["x_hat"] for c in range(NCORES)])
    return coeffs, p_logits, p_hard, x_hat
